# revision 40
# baseline (speedup 1.0000x reference)
"""Trainium2 Bass kernel for DynamicGraphAttention.

Computation (per batch b of 256):
    xs = x[b, 0]                          # (64, 4096)
    e  = var(xs, axis=-1, ddof=1)         # (64,)
    Q  = wq[:,None] * e[None,:] + bq[:,None]   # (16, 64)
    K  = wk[:,None] * e[None,:] + bk[:,None]
    L  = Q^T K * SCALE                    # (64, 64) == a*e_i*e_j + b*e_i + c*e_j + d
    L[diag] = -1e9 ; W = softmax(L, axis=-1)
    out = W @ xs + xs ; return out, W

Key algebraic reduction: the 16-dim channel contraction collapses to 4 host
scalars a,b,c,d (products of the tiny parameter vectors), so on-device work is
variance (bn_stats) + a rank-2 matmul for logits + softmax + one 64-contract
matmul per batch. The "+ xs" is folded into the matmul by adding the identity
to the softmax weights; two batches are packed per 128-partition tile and
their weight matrices form a 128x128 block-diagonal so the big matmul runs
both batches in one pass.

Sharding: pure data parallel, 32 batches per core across 8 cores.
"""

import numpy as np

import concourse.bacc as bacc
import concourse.tile as tile
from concourse import mybir
from concourse.bass_utils import run_bass_kernel_spmd

F32 = mybir.dt.float32

N_CORES = 8
B_TOTAL = 256
B_PER_CORE = B_TOTAL // N_CORES  # 32
N_NODES = 64
T = 4096
HIDDEN = 16
SCALE = HIDDEN ** -0.5
NEG_INF = -1e9
CHUNK = 512
N_CHUNKS = T // CHUNK  # 8


def build_nc(n_pairs=B_PER_CORE // 2):
    """Build the per-core Bass program. Each pair = 2 batches = 128 rows."""
    rows = n_pairs * 128
    nc = bacc.Bacc("TRN2", target_bir_lowering=False, debug=False)

    x_d = nc.dram_tensor("x", [rows, T], F32, kind="ExternalInput").ap()
    coef_d = nc.dram_tensor("coef", [1, 4], F32, kind="ExternalInput").ap()
    ident_d = nc.dram_tensor("ident", [128, 128], F32, kind="ExternalInput").ap()
    negeye_d = nc.dram_tensor("negeye", [128, 64], F32, kind="ExternalInput").ap()
    out_d = nc.dram_tensor("out", [rows, T], F32, kind="ExternalOutput").ap()
    attn_d = nc.dram_tensor("attn", [rows, 64], F32, kind="ExternalOutput").ap()

    with tile.TileContext(nc) as tc:
        with (
            tc.tile_pool(name="consts", bufs=1) as consts,
            tc.tile_pool(name="xs_pool", bufs=6) as xs_pool,
            tc.tile_pool(name="out_pool", bufs=4) as out_pool,
            tc.tile_pool(name="small", bufs=8) as small,
            tc.tile_pool(name="bd_pool", bufs=4) as bd_pool,
            tc.tile_pool(name="ps_small", bufs=2, space="PSUM") as ps_small,
            tc.tile_pool(name="ps_big", bufs=2, space="PSUM") as ps_big,
        ):
            ident = consts.tile([128, 128], F32)
            nc.sync.dma_start(out=ident, in_=ident_d)
            negeye = consts.tile([128, 64], F32)
            nc.sync.dma_start(out=negeye, in_=negeye_d)
            coef = consts.tile([1, 4], F32)
            nc.sync.dma_start(out=coef, in_=coef_d)
            ones1 = consts.tile([1, 128], F32)
            nc.vector.memset(ones1, 1.0)

            live = {}
            live2 = {}

            def stage1(p):
                """Load pair p, compute per-row variance."""
                xs = xs_pool.tile([128, T], F32, name=f"xs{p}", tag="xs")
                xs_r = xs.rearrange("p (c f) -> p c f", f=nc.vector.BN_STATS_FMAX)
                nsub = xs_r.shape[1]
                stats = small.tile([128, nsub, nc.vector.BN_STATS_DIM], F32,
                                   name=f"stats{p}", tag="stats")
                nc.sync.dma_start(out=xs, in_=x_d[p * 128:(p + 1) * 128, :])
                for c in range(nsub):
                    nc.vector.bn_stats(out=stats[:, c, :], in_=xs_r[:, c, :])
                mv = small.tile([128, 2], F32, name=f"mv{p}", tag="mv")
                nc.vector.bn_aggr(out=mv, in_=stats)
                evar = mv[:, 1:2]  # biased variance, (128, 1)

                # L[i,j] = a e_i e_j + b e_i + c e_j + d per batch:
                # outer(e, s0) + outer(1, s1) via accumulating 1-contract matmuls
                eT_ps = ps_small.tile([1, 128], F32, name=f"eT{p}", tag="eT")
                nc.tensor.transpose(eT_ps, evar, ident)
                e_row = small.tile([1, 128], F32, name=f"e_row{p}", tag="e_row")
                nc.scalar.copy(e_row, eT_ps)
                s0 = small.tile([1, 128], F32, name=f"s0{p}", tag="s0")
                nc.scalar.activation(s0, e_row, mybir.ActivationFunctionType.Identity,
                                     bias=coef[:, 1:2], scale=coef[:, 0:1])
                s1 = small.tile([1, 128], F32, name=f"s1{p}", tag="s1")
                nc.scalar.activation(s1, e_row, mybir.ActivationFunctionType.Identity,
                                     bias=coef[:, 3:4], scale=coef[:, 2:3])
                L_ps = ps_small.tile([128, 64], F32, name=f"L{p}", tag="L")
                nc.tensor.matmul(L_ps[0:64, :], lhsT=e_row[:, 0:64],
                                 rhs=s0[:, 0:64], start=True, stop=False)
                nc.tensor.matmul(L_ps[0:64, :], lhsT=ones1[:, 0:64],
                                 rhs=s1[:, 0:64], start=False, stop=True)
                nc.tensor.matmul(L_ps[64:128, :], lhsT=e_row[:, 64:128],
                                 rhs=s0[:, 64:128], start=True, stop=False)
                nc.tensor.matmul(L_ps[64:128, :], lhsT=ones1[:, 64:128],
                                 rhs=s1[:, 64:128], start=False, stop=True)
                live[p] = (xs, L_ps)

            def stage2a(p):
                """Softmax, weight transpose -> bd."""
                xs, L_ps = live.pop(p)
                r0 = p * 128
                # softmax along free axis, diagonal masked.
                # negLm = -(L + negeye); mneg = min(negLm) = -max(L + negeye)
                Lm = small.tile([128, 64], F32, name=f"Lm{p}", tag="Lm")
                nc.vector.tensor_add(Lm, L_ps, negeye)
                mneg = small.tile([128, 1], F32, name=f"mneg{p}", tag="mneg")
                nc.vector.reduce_max(mneg, Lm, axis=mybir.AxisListType.X, negate=True)
                E = small.tile([128, 64], F32, name=f"E{p}", tag="E")
                dsum = small.tile([128, 1], F32, name=f"dsum{p}", tag="dsum")
                nc.scalar.activation(E, Lm, mybir.ActivationFunctionType.Exp,
                                     bias=mneg, scale=1.0, accum_out=dsum)
                r_t = small.tile([128, 1], F32, name=f"r{p}", tag="r")
                nc.vector.reciprocal(r_t, dsum)
                Wn = small.tile([128, 64], F32, name=f"Wn{p}", tag="Wn")
                nc.scalar.mul(Wn, E, r_t)
                nc.gpsimd.dma_start(out=attn_d[r0:r0 + 128, :], in_=Wn)

                # block-diag of Wn, one full 128x128 PE transpose, then +I128
                wbd = bd_pool.tile([128, 128], F32, name=f"wbd{p}", tag="wbd")
                nc.gpsimd.memset(wbd, 0.0)
                nc.gpsimd.tensor_copy(wbd[0:64, 0:64], Wn[0:64, :])
                nc.gpsimd.tensor_copy(wbd[64:128, 64:128], Wn[64:128, :])
                bdT_ps = ps_small.tile([128, 128], F32, name=f"bdT{p}",
                                       tag="bdT", bufs=2)
                nc.tensor.transpose(bdT_ps, wbd, ident)
                bd = bd_pool.tile([128, 128], F32, name=f"bd{p}", tag="bd")
                nc.vector.tensor_add(bd, bdT_ps, ident)
                live2[p] = (xs, bd)

            def stage2b(p):
                """Big matmul, drains, stores."""
                xs, bd = live2.pop(p)
                r0 = p * 128
                # out = (W + I) @ xs, chunked over the free dim
                out_sb = out_pool.tile([128, T], F32, name=f"out_sb{p}", tag="out_sb")
                for c in range(N_CHUNKS):
                    pc = ps_big.tile([128, CHUNK], F32, name=f"pc{p}_{c}", tag="psb")
                    nc.tensor.matmul(pc, lhsT=bd, rhs=xs[:, c * CHUNK:(c + 1) * CHUNK],
                                     start=True, stop=True)
                    nc.scalar.copy(out_sb[:, c * CHUNK:(c + 1) * CHUNK], pc)
                    if c % 2 == 1:
                        q0, q1 = (c - 1) * CHUNK, (c + 1) * CHUNK
                        nc.scalar.dma_start(out=out_d[r0:r0 + 128, q0:q1],
                                            in_=out_sb[:, q0:q1])

            for it in range(n_pairs + 2):
                if 1 <= it <= n_pairs:
                    stage2a(it - 1)
                if it < n_pairs:
                    stage1(it)
                if it >= 2:
                    stage2b(it - 2)

    nc.compile()
    return nc


def host_consts(wq, bq, wk, bk):
    k = T / (T - 1.0)  # unbiased-variance correction, folded into the coefs
    wq = wq.astype(np.float64); bq = bq.astype(np.float64)
    wk = wk.astype(np.float64); bk = bk.astype(np.float64)
    a = SCALE * float(wq @ wk) * k * k
    b = SCALE * float(wq @ bk) * k
    c = SCALE * float(bq @ wk) * k
    d = SCALE * float(bq @ bk)
    coef = np.array([[a, b, c, d]], dtype=np.float32)
    ident = np.eye(128, dtype=np.float32)
    negeye = np.zeros((128, 64), dtype=np.float32)
    negeye[np.arange(128), np.arange(128) % 64] = NEG_INF
    return coef, ident, negeye


_NC_CACHE = {}


def _get_nc():
    if "nc" not in _NC_CACHE:
        _NC_CACHE["nc"] = build_nc()
    return _NC_CACHE["nc"]


def run_on_hw(x, wq, bq, wk, bk, trace=False, **kw):
    """Shard, run the SPMD kernel on 8 cores, gather. Returns (out, attn, results)."""
    nc = _get_nc()
    coef, ident, negeye = host_consts(wq, bq, wk, bk)
    xf = np.ascontiguousarray(x.reshape(B_TOTAL * N_NODES, T))
    rows_pc = B_PER_CORE * N_NODES
    in_maps = [
        {"x": xf[k * rows_pc:(k + 1) * rows_pc], "coef": coef,
         "ident": ident, "negeye": negeye}
        for k in range(N_CORES)
    ]
    res = None
    for attempt in range(3):
        try:
            res = run_bass_kernel_spmd(nc, in_maps, list(range(N_CORES)),
                                       trace=trace, **kw)
            break
        except Exception:
            if attempt == 2:
                raise
            # Transient NRT "exec unit unrecoverable" faults have been
            # observed; reset the jax backend so the next attempt gets a
            # fresh client instead of the poisoned one.
            try:
                import jax
                jax.clear_backends()
            except Exception:
                pass
    assert res is not None
    out = np.concatenate([res.results[k]["out"] for k in range(N_CORES)], axis=0)
    attn = np.concatenate([res.results[k]["attn"] for k in range(N_CORES)], axis=0)
    out = out.reshape(B_TOTAL, 1, N_NODES, T)
    attn = attn.reshape(B_TOTAL, N_NODES, N_NODES)
    return out, attn, res


def kernel(x, wq, bq, wk, bk):
    out, attn, _ = run_on_hw(np.asarray(x), np.asarray(wq), np.asarray(bq),
                             np.asarray(wk), np.asarray(bk))
    return out, attn


# revision 42
# speedup vs baseline: 1.1869x; 1.1869x over previous
"""Trainium2 Bass kernel for DynamicGraphAttention.

Computation (per batch b of 256):
    xs = x[b, 0]                          # (64, 4096)
    e  = var(xs, axis=-1, ddof=1)         # (64,)
    Q  = wq[:,None] * e[None,:] + bq[:,None]   # (16, 64)
    K  = wk[:,None] * e[None,:] + bk[:,None]
    L  = Q^T K * SCALE                    # (64, 64) == a*e_i*e_j + b*e_i + c*e_j + d
    L[diag] = -1e9 ; W = softmax(L, axis=-1)
    out = W @ xs + xs ; return out, W

Key algebraic reduction: the 16-dim channel contraction collapses to 4 host
scalars a,b,c,d (products of the tiny parameter vectors), so on-device work is
variance (bn_stats) + a rank-2 matmul for logits + softmax + one 64-contract
matmul per batch. The "+ xs" is folded into the matmul by adding the identity
to the softmax weights; two batches are packed per 128-partition tile and
their weight matrices form a 128x128 block-diagonal so the big matmul runs
both batches in one pass.

Sharding: pure data parallel, 32 batches per core across 8 cores.
"""

import numpy as np

import concourse.bacc as bacc
import concourse.tile as tile
from concourse import mybir
from concourse.bass_utils import run_bass_kernel_spmd

F32 = mybir.dt.float32

N_CORES = 8
B_TOTAL = 256
B_PER_CORE = B_TOTAL // N_CORES  # 32
N_NODES = 64
T = 4096
HIDDEN = 16
SCALE = HIDDEN ** -0.5
NEG_INF = -1e9
CHUNK = 512
N_CHUNKS = T // CHUNK  # 8


def build_nc(n_pairs=B_PER_CORE // 2):
    """Build the per-core Bass program. Each pair = 2 batches = 128 rows."""
    rows = n_pairs * 128
    nc = bacc.Bacc("TRN2", target_bir_lowering=False, debug=False)

    x_d = nc.dram_tensor("x", [rows, T], F32, kind="ExternalInput").ap()
    coef_d = nc.dram_tensor("coef", [1, 4], F32, kind="ExternalInput").ap()
    ident_d = nc.dram_tensor("ident", [128, 128], F32, kind="ExternalInput").ap()
    negeye_d = nc.dram_tensor("negeye", [128, 64], F32, kind="ExternalInput").ap()
    out_d = nc.dram_tensor("out", [rows, T], F32, kind="ExternalOutput").ap()
    attn_d = nc.dram_tensor("attn", [rows, 64], F32, kind="ExternalOutput").ap()

    with tile.TileContext(nc) as tc:
        with (
            tc.tile_pool(name="consts", bufs=1) as consts,
            tc.tile_pool(name="xs_pool", bufs=6) as xs_pool,
            tc.tile_pool(name="out_pool", bufs=4) as out_pool,
            tc.tile_pool(name="small", bufs=8) as small,
            tc.tile_pool(name="bd_pool", bufs=4) as bd_pool,
            tc.tile_pool(name="ps_small", bufs=2, space="PSUM") as ps_small,
            tc.tile_pool(name="ps_big", bufs=2, space="PSUM") as ps_big,
        ):
            ident = consts.tile([128, 128], F32)
            nc.sync.dma_start(out=ident, in_=ident_d)
            negeye = consts.tile([128, 64], F32)
            nc.sync.dma_start(out=negeye, in_=negeye_d)
            coef = consts.tile([1, 4], F32)
            nc.sync.dma_start(out=coef, in_=coef_d)
            ones1 = consts.tile([1, 128], F32)
            nc.vector.memset(ones1, 1.0)

            live = {}
            live2 = {}

            def stage1(p):
                """Load pair p, compute per-row variance."""
                xs = xs_pool.tile([128, T], F32, name=f"xs{p}", tag="xs")
                xs_r = xs.rearrange("p (c f) -> p c f", f=nc.vector.BN_STATS_FMAX)
                nsub = xs_r.shape[1]
                stats = small.tile([128, nsub, nc.vector.BN_STATS_DIM], F32,
                                   name=f"stats{p}", tag="stats")
                NQ = 4 if p < 2 else 1  # quarter-load the first pairs: the
                # pipeline head is the only place load latency is exposed
                for h in range(NQ):
                    cols = slice(h * T // NQ, (h + 1) * T // NQ)
                    nc.sync.dma_start(out=xs[:, cols],
                                      in_=x_d[p * 128:(p + 1) * 128, cols])
                    for c in range(h * nsub // NQ, (h + 1) * nsub // NQ):
                        nc.vector.bn_stats(out=stats[:, c, :], in_=xs_r[:, c, :])
                mv = small.tile([128, 2], F32, name=f"mv{p}", tag="mv")
                nc.vector.bn_aggr(out=mv, in_=stats)
                live[p] = (xs, mv)

            def stage2a(p):
                """Logits, softmax, weight transpose -> bd."""
                xs, mv = live.pop(p)
                r0 = p * 128
                evar = mv[:, 1:2]  # biased variance, (128, 1)

                # L[i,j] = a e_i e_j + b e_i + c e_j + d per batch:
                # outer(e, s0) + outer(1, s1) via accumulating 1-contract matmuls
                eT_ps = ps_small.tile([1, 128], F32, name=f"eT{p}", tag="eT")
                nc.tensor.transpose(eT_ps, evar, ident)
                e_row = small.tile([1, 128], F32, name=f"e_row{p}", tag="e_row")
                nc.scalar.copy(e_row, eT_ps)
                s0 = small.tile([1, 128], F32, name=f"s0{p}", tag="s0")
                nc.scalar.activation(s0, e_row, mybir.ActivationFunctionType.Identity,
                                     bias=coef[:, 1:2], scale=coef[:, 0:1])
                s1 = small.tile([1, 128], F32, name=f"s1{p}", tag="s1")
                nc.scalar.activation(s1, e_row, mybir.ActivationFunctionType.Identity,
                                     bias=coef[:, 3:4], scale=coef[:, 2:3])
                L_ps = ps_small.tile([128, 64], F32, name=f"L{p}", tag="L")
                nc.tensor.matmul(L_ps[0:64, :], lhsT=e_row[:, 0:64],
                                 rhs=s0[:, 0:64], start=True, stop=False)
                nc.tensor.matmul(L_ps[0:64, :], lhsT=ones1[:, 0:64],
                                 rhs=s1[:, 0:64], start=False, stop=True)
                nc.tensor.matmul(L_ps[64:128, :], lhsT=e_row[:, 64:128],
                                 rhs=s0[:, 64:128], start=True, stop=False)
                nc.tensor.matmul(L_ps[64:128, :], lhsT=ones1[:, 64:128],
                                 rhs=s1[:, 64:128], start=False, stop=True)

                # softmax along free axis, diagonal masked.
                # negLm = -(L + negeye); mneg = min(negLm) = -max(L + negeye)
                Lm = small.tile([128, 64], F32, name=f"Lm{p}", tag="Lm")
                nc.vector.tensor_add(Lm, L_ps, negeye)
                mneg = small.tile([128, 1], F32, name=f"mneg{p}", tag="mneg")
                nc.vector.reduce_max(mneg, Lm, axis=mybir.AxisListType.X, negate=True)
                E = small.tile([128, 64], F32, name=f"E{p}", tag="E")
                dsum = small.tile([128, 1], F32, name=f"dsum{p}", tag="dsum")
                nc.scalar.activation(E, Lm, mybir.ActivationFunctionType.Exp,
                                     bias=mneg, scale=1.0, accum_out=dsum)
                r_t = small.tile([128, 1], F32, name=f"r{p}", tag="r")
                nc.vector.reciprocal(r_t, dsum)
                Wn = small.tile([128, 64], F32, name=f"Wn{p}", tag="Wn")
                nc.scalar.mul(Wn, E, r_t)
                nc.gpsimd.dma_start(out=attn_d[r0:r0 + 128, :], in_=Wn)

                # block-diag of Wn, one full 128x128 PE transpose, then +I128
                wbd = bd_pool.tile([128, 128], F32, name=f"wbd{p}", tag="wbd")
                nc.gpsimd.memset(wbd, 0.0)
                nc.gpsimd.tensor_copy(wbd[0:64, 0:64], Wn[0:64, :])
                nc.gpsimd.tensor_copy(wbd[64:128, 64:128], Wn[64:128, :])
                bdT_ps = ps_small.tile([128, 128], F32, name=f"bdT{p}",
                                       tag="bdT", bufs=2)
                nc.tensor.transpose(bdT_ps, wbd, ident)
                bd = bd_pool.tile([128, 128], F32, name=f"bd{p}", tag="bd")
                nc.vector.tensor_add(bd, bdT_ps, ident)
                live2[p] = (xs, bd)

            def stage2b(p):
                """Big matmul, drains, stores."""
                xs, bd = live2.pop(p)
                r0 = p * 128
                # out = (W + I) @ xs, chunked over the free dim
                out_sb = out_pool.tile([128, T], F32, name=f"out_sb{p}", tag="out_sb")
                for c in range(N_CHUNKS):
                    pc = ps_big.tile([128, CHUNK], F32, name=f"pc{p}_{c}", tag="psb")
                    nc.tensor.matmul(pc, lhsT=bd, rhs=xs[:, c * CHUNK:(c + 1) * CHUNK],
                                     start=True, stop=True)
                    nc.scalar.copy(out_sb[:, c * CHUNK:(c + 1) * CHUNK], pc)
                    if c % 2 == 1:
                        q0, q1 = (c - 1) * CHUNK, (c + 1) * CHUNK
                        nc.scalar.dma_start(out=out_d[r0:r0 + 128, q0:q1],
                                            in_=out_sb[:, q0:q1])

            for it in range(n_pairs + 2):
                if 1 <= it <= n_pairs:
                    stage2a(it - 1)
                if it < n_pairs:
                    stage1(it)
                if it >= 2:
                    stage2b(it - 2)

    nc.compile()
    return nc


def host_consts(wq, bq, wk, bk):
    k = T / (T - 1.0)  # unbiased-variance correction, folded into the coefs
    wq = wq.astype(np.float64); bq = bq.astype(np.float64)
    wk = wk.astype(np.float64); bk = bk.astype(np.float64)
    a = SCALE * float(wq @ wk) * k * k
    b = SCALE * float(wq @ bk) * k
    c = SCALE * float(bq @ wk) * k
    d = SCALE * float(bq @ bk)
    coef = np.array([[a, b, c, d]], dtype=np.float32)
    ident = np.eye(128, dtype=np.float32)
    negeye = np.zeros((128, 64), dtype=np.float32)
    negeye[np.arange(128), np.arange(128) % 64] = NEG_INF
    return coef, ident, negeye


_NC_CACHE = {}


def _get_nc():
    if "nc" not in _NC_CACHE:
        _NC_CACHE["nc"] = build_nc()
    return _NC_CACHE["nc"]


def run_on_hw(x, wq, bq, wk, bk, trace=False, **kw):
    """Shard, run the SPMD kernel on 8 cores, gather. Returns (out, attn, results)."""
    nc = _get_nc()
    coef, ident, negeye = host_consts(wq, bq, wk, bk)
    xf = np.ascontiguousarray(x.reshape(B_TOTAL * N_NODES, T))
    rows_pc = B_PER_CORE * N_NODES
    in_maps = [
        {"x": xf[k * rows_pc:(k + 1) * rows_pc], "coef": coef,
         "ident": ident, "negeye": negeye}
        for k in range(N_CORES)
    ]
    res = None
    for attempt in range(3):
        try:
            res = run_bass_kernel_spmd(nc, in_maps, list(range(N_CORES)),
                                       trace=trace, **kw)
            break
        except Exception:
            if attempt == 2:
                raise
            # Transient NRT "exec unit unrecoverable" faults have been
            # observed; reset the jax backend so the next attempt gets a
            # fresh client instead of the poisoned one.
            try:
                import jax
                jax.clear_backends()
            except Exception:
                pass
    assert res is not None
    out = np.concatenate([res.results[k]["out"] for k in range(N_CORES)], axis=0)
    attn = np.concatenate([res.results[k]["attn"] for k in range(N_CORES)], axis=0)
    out = out.reshape(B_TOTAL, 1, N_NODES, T)
    attn = attn.reshape(B_TOTAL, N_NODES, N_NODES)
    return out, attn, res


def kernel(x, wq, bq, wk, bk):
    out, attn, _ = run_on_hw(np.asarray(x), np.asarray(wq), np.asarray(bq),
                             np.asarray(wk), np.asarray(bk))
    return out, attn
